# revision 14
# baseline (speedup 1.0000x reference)
"""AdapterGNN distributed Trainium2 kernel (8 NeuronCores, Bass/Tile).

out = norm_dst * segsum_dst( ((X*norm_src) @ Wd + norm_src*bd)[src] ) @ (Wg@Wu) + (bg@Wu+bu)

Sharding (src-side aggregation + ReduceScatter):
  Nodes are split contiguously across 8 cores. Each core down-projects its own
  node block (h, fp16) into a LOCAL DRAM table, then processes the edges whose
  SRC it owns: per-edge h rows are fetched with batched dma_gather instructions
  from the local table (no pre-collective!), and a PE segment-sum accumulates
  per-dst-window partial aggregates: for each 128-edge block, a selection
  matrix S[e, d] = is_equal(iota, slot_e) is built on the DVE and
  psum[f, d] += G_blk^T-contract-S.  Partials are drained FEATURE-major
  ([128 f, dst]) so the DRAM writes are >=1792B runs (full DMA bandwidth).

  The global dst space (8 chunks x 98 windows) is processed in 7 "pieces"
  (14 windows of every chunk per piece); after each piece a ReduceScatter(add)
  combines the 8 cores' partials and hands each core its own dst rows --
  7 small collectives (~26.5us each) pipelined behind the gather stream
  instead of one 284us AllGather blocking everything.

  Per-piece tail: the RS output (aggT, f-major = exactly the up-projection's
  lhsT layout) is loaded back, multiplied by the fused (Wg@Wu) weight, and the
  Activation-engine psum drain applies norm_dst as its per-partition scale.
  norm_src and bd ride in the down-projection inputs (host-folded); bg@Wu+bu
  is added on the host.

Self-contained: requires only numpy + concourse (+ TRN2 cores via axon).
"""

import numpy as np

import concourse.bacc as bacc
import concourse.bass as bass
import concourse.mybir as mybir
import concourse.tile as tile
from concourse import library_config
from concourse.bass_utils import run_bass_kernel_spmd

F32 = mybir.dt.float32
F16 = mybir.dt.float16
I16 = mybir.dt.int16

P = 128           # partitions
WPG = 7           # windows per psum group ((WPG+1)*128 f32 = 2 psum banks)
PIECE_G = (7, 6, 1)   # groups-of-7 per (piece, chunk): 49+42+7 = 98 windows
MAXH = 4          # max windows one 128-edge block may span
IOTA_W = MAXH * P
GATHER_MAX = 10240    # target idxs per gather instruction


class Cfg:
    def __init__(self, n_nodes, n_edges, in_dim, out_dim, n_cores=8):
        self.N = n_nodes
        self.E = n_edges
        self.IN = in_dim          # 768
        self.OUT = out_dim        # 128 (must be 128)
        self.C = n_cores
        assert out_dim == P
        self.NpReal = (n_nodes + n_cores - 1) // n_cores   # real nodes per core
        self.W = (self.NpReal + P - 1) // P                # windows per chunk
        assert self.W == sum(PIECE_G) * WPG, self.W
        self.PIECES = len(PIECE_G)
        self.Np = self.W * P                               # padded nodes/chunk
        self.KC = in_dim // P                              # full K chunks (6)
        assert in_dim % P == 0
        self.KIN = self.KC + 1                             # +1 chunk: (norm,bias) row
        self.GPC = sum(PIECE_G)                            # groups per chunk (14)
        self.NSG = n_cores * self.GPC                      # supergroups (112)
        # window offset of each piece within a chunk, in windows
        self.piece_w0 = [WPG * sum(PIECE_G[:k]) for k in range(self.PIECES)]
        self.piece_nw = [WPG * g for g in PIECE_G]


def _ceil128(x):
    return int(-(-int(x) // 128) * 128)


def host_prep(cfg, features, Wd, bd, Wg, bg, Wu, bu, src, dst):
    """Returns (in_maps, node_core, node_slot, prof)."""
    C, N, Np, W = cfg.C, cfg.N, cfg.Np, cfg.W
    src = np.asarray(src).astype(np.int64)
    dst = np.asarray(dst).astype(np.int64)
    features = np.asarray(features, dtype=np.float32)

    out_deg = np.bincount(src, minlength=N)
    in_deg = np.bincount(dst, minlength=N)
    norm_src = 1.0 / np.sqrt(np.maximum(out_deg, 1.0))
    norm_dst = 1.0 / np.sqrt(np.maximum(in_deg, 1.0))

    nodes = np.arange(N)
    node_core = np.minimum(nodes // cfg.NpReal, C - 1)
    node_slot = nodes - node_core * cfg.NpReal

    # per-edge decomposition; edge owned by src's core
    ecore = node_core[src]
    erow = node_slot[src]                      # local table row of h[src]
    dchunk = node_core[dst]
    dloc = node_slot[dst]
    dwc = dloc // P                            # window within dst chunk (0..97)
    dp = dloc % P
    gc = dwc // WPG                            # group within chunk (0..13)
    wig = dwc % WPG                            # window in group (0..6)
    # piece of each group-of-7 and group index within the piece
    gcum = np.cumsum((0,) + PIECE_G)           # [0, 7, 13, 14]
    kp_of_g = np.searchsorted(gcum[1:], np.arange(cfg.GPC), side="right")
    gip_of_g = np.arange(cfg.GPC) - gcum[kp_of_g]
    kp = kp_of_g[gc]                           # piece (0..2)
    gi = gip_of_g[gc]                          # group in (piece, chunk)
    # supergroup = program-order group index: by (piece, chunk, group)
    sg_base = np.concatenate([[0], np.cumsum([PIECE_G[k] * C for k in range(cfg.PIECES)])])
    sg = sg_base[kp] + dchunk * np.asarray(PIECE_G)[kp] + gi
    NSG = cfg.NSG

    # shared per-(sg, window) maxima
    NC = np.zeros((C, NSG, WPG), dtype=np.int64)
    np.add.at(NC, (ecore, sg, wig), 1)
    M = NC.max(axis=0)                         # [NSG, WPG]

    # blocks per supergroup chunk (stream ceil128 per sg)
    chunk_off = np.zeros(NSG, dtype=np.int64)
    chunk_len = np.zeros(NSG, dtype=np.int64)
    blocks = []       # (sg, k_in_chunk, lo, nh, bi)
    off = 0
    for s in range(NSG):
        seg = M[s]
        L = _ceil128(seg.sum())
        chunk_off[s] = off
        chunk_len[s] = L
        bcum = np.concatenate([[0], np.cumsum(seg)])
        for k in range(L // 128):
            p0, p1 = k * 128, k * 128 + 127
            lo = int(np.searchsorted(bcum[1:], p0, side="right"))
            hi = int(np.searchsorted(bcum[1:], p1, side="right"))
            lo, hi = min(lo, WPG - 1), min(hi, WPG - 1)
            nh = hi - lo + 1
            assert nh <= MAXH, f"block spans {nh} windows"
            blocks.append((s, k, lo, nh, len(blocks)))
        off += L
    T = int(off)
    NBLK = len(blocks)
    assert T == NBLK * 128

    # psum zero-region start/stop flags per (sg, 2KB region = ws//4)
    first_mm = {}
    last_mm = {}
    for s, k, lo, nh, bi in blocks:
        for h in range(nh):
            key2 = (s, (lo + h) // 4)
            if key2 not in first_mm:
                first_mm[key2] = (bi, h)
            last_mm[key2] = (bi, h)

    # gather instruction cells: greedy-pack consecutive sgs (never across pieces)
    sg_base_l = [0]
    for k in range(cfg.PIECES):
        sg_base_l.append(sg_base_l[-1] + PIECE_G[k] * C)
    gathers = []          # (o16, L)
    gather_of_sg = {}     # sg -> (cell idx, o16*16//128 = first block idx)
    piece_cells = []      # per piece: list of cell indices
    for k in range(cfg.PIECES):
        cells = []
        s0 = sg_base_l[k]
        send = sg_base_l[k + 1]
        cur = s0
        while cur < send:
            o = int(chunk_off[cur])
            L = 0
            first = cur
            while cur < send and (L == 0 or L + chunk_len[cur] <= GATHER_MAX):
                L += int(chunk_len[cur])
                cur += 1
            ci = len(gathers)
            gathers.append((o // 16, L))
            for s2 in range(first, cur):
                gather_of_sg[s2] = (ci, o // 128)
            cells.append(ci)
        piece_cells.append(tuple(cells))

    prof = {
        "chunk_off": tuple(int(x) for x in chunk_off),
        "chunk_len": tuple(int(x) for x in chunk_len),
        "blocks": tuple(blocks),
        "first": frozenset(first_mm.items()),
        "last": frozenset(last_mm.items()),
        "gathers": tuple(gathers),
        "gather_of_sg": tuple(sorted(gather_of_sg.items())),
        "piece_cells": tuple(tuple(x) for x in piece_cells),
        "sg_base": tuple(sg_base_l),
        "T": T,
        "NBLK": NBLK,
    }

    # fused weights
    Wgu = (np.asarray(Wg, np.float64) @ np.asarray(Wu, np.float64)).astype(np.float32)
    bu2 = (np.asarray(bg, np.float64) @ np.asarray(Wu, np.float64) + bu).astype(np.float32)

    wd_h = np.zeros((P, cfg.KIN * cfg.OUT), dtype=np.float16)
    for cc in range(cfg.KC):
        wd_h[:, cc * cfg.OUT:(cc + 1) * cfg.OUT] = Wd[cc * P:(cc + 1) * P, :]
    wd_h[0, cfg.KC * cfg.OUT:(cfg.KC + 1) * cfg.OUT] = bd
    wgu_h = Wgu.astype(np.float16)

    consts = np.zeros((P, IOTA_W), dtype=np.float16)
    consts[:, :] = np.arange(IOTA_W, dtype=np.float16)[None, :]

    # per-block lo for slot_rel
    blk_lo = np.zeros(NBLK, dtype=np.int64)
    for s, k, lo, nh, bi in blocks:
        assert chunk_off[s] // 128 + k == bi
        blk_lo[bi] = lo

    # intra-chunk window segment offsets (shared)
    segoff = np.zeros((NSG, WPG), dtype=np.int64)
    for s in range(NSG):
        segoff[s] = chunk_off[s] + np.concatenate([[0], np.cumsum(M[s])[:-1]])

    S16 = T // 16
    in_maps = []
    for c in range(C):
        em = np.where(ecore == c)[0]
        sgc, wigc, dpc = sg[em], wig[em], dp[em]
        order = np.lexsort((dpc, wigc, sgc))
        em, sgc, wigc, dpc = em[order], sgc[order], wigc[order], dpc[order]
        segid = sgc * WPG + wigc
        uniq, counts = np.unique(segid, return_counts=True)
        cum = np.concatenate([[0], np.cumsum(counts)])
        rank = np.arange(len(em)) - cum[np.searchsorted(uniq, segid)]
        pos = segoff[sgc, wigc] + rank
        assert (rank < M[sgc, wigc]).all()

        idx_s = np.zeros(T, dtype=np.int64)
        slotg = np.full(T, -1, dtype=np.int64)   # slot within group (wig*128+p)
        idx_s[pos] = erow[em]
        slotg[pos] = wigc * P + dpc

        # per-block relative slots
        slot_rel = slotg.reshape(-1, 128) - blk_lo[:, None] * P
        slot_rel[slotg.reshape(-1, 128) < 0] = -1
        assert (slot_rel < MAXH * P).all()

        idx16 = np.zeros((P, S16), dtype=np.int16)
        sidx = np.arange(S16) * 16
        for p in range(P):
            idx16[p, :] = idx_s[sidx + (p % 16)]
        slotv = np.ascontiguousarray(slot_rel.T.astype(np.float32))   # [128, NBLK]

        # xa: window-blocked [p, w*KIN*128 + cc*128 + n]; own nodes, norm_src folded
        nt_ids = np.where(node_core == np.int64(c))[0]
        xs = (features[nt_ids, :] * norm_src[nt_ids, None]).astype(np.float16)
        xa = np.zeros((P, W * cfg.KIN * P), dtype=np.float16)
        xs_slot = np.zeros((Np, cfg.IN), dtype=np.float16)
        xs_slot[node_slot[nt_ids], :] = xs
        nsr = np.zeros(Np, dtype=np.float16)
        nsr[node_slot[nt_ids]] = norm_src[nt_ids].astype(np.float16)
        for w in range(W):
            blkb = w * cfg.KIN * P
            rows = xs_slot[w * P:(w + 1) * P, :]
            for cc in range(cfg.KC):
                xa[:, blkb + cc * P:blkb + (cc + 1) * P] = rows[:, cc * P:(cc + 1) * P].T
            xa[0, blkb + cfg.KC * P:blkb + (cfg.KC + 1) * P] = nsr[w * P:(w + 1) * P]

        # own-chunk norm_dst per (partition, window)
        ndstw = np.zeros((P, W), dtype=np.float32)
        nd = np.zeros(Np, dtype=np.float32)
        nd[node_slot[nt_ids]] = norm_dst[nt_ids].astype(np.float32)
        ndstw[:, :] = nd.reshape(W, P).T

        in_maps.append(
            {
                "xa": xa,
                "idx": idx16,
                "slotv": slotv,
                "wd": wd_h,
                "wgu": wgu_h,
                "consts": consts,
                "ndstw": ndstw,
            }
        )

    return in_maps, node_core, node_slot, prof


def build_graph(cfg, prof):
    """Build the SPMD Bass graph (same for all cores)."""
    W, OUT, IN = cfg.W, cfg.OUT, cfg.IN
    C, PIECES = cfg.C, cfg.PIECES
    blocks = prof["blocks"]
    first_mm = dict(prof["first"])
    last_mm = dict(prof["last"])
    gathers = prof["gathers"]
    gather_of_sg = dict(prof["gather_of_sg"])
    piece_cells = prof["piece_cells"]
    sg_base = prof["sg_base"]
    T = prof["T"]
    NBLK = prof["NBLK"]
    S16 = T // 16

    blocks_by_sg = {}
    for b in blocks:
        blocks_by_sg.setdefault(b[0], []).append(b)

    nc = bacc.Bacc(None, target_bir_lowering=False)
    xa = nc.declare_dram_parameter("xa", [P, W * cfg.KIN * P], F16, False)
    idx = nc.declare_dram_parameter("idx", [P, S16], I16, False)
    slotv = nc.declare_dram_parameter("slotv", [P, NBLK], F32, False)
    wd = nc.declare_dram_parameter("wd", [P, cfg.KIN * OUT], F16, False)
    wgu = nc.declare_dram_parameter("wgu", [OUT, IN], F16, False)
    consts = nc.declare_dram_parameter("consts", [P, IOTA_W], F16, False)
    ndstw = nc.declare_dram_parameter("ndstw", [P, W], F32, False)
    out = nc.declare_dram_parameter("out", [cfg.Np, IN], F16, True)

    with tile.TileContext(nc) as tc:
        with (
            tc.tile_pool(name="dram", bufs=1, space="DRAM") as dram,
            tc.tile_pool(name="gpsum", bufs=2, space="PSUM") as gpsum,
            tc.tile_pool(name="upsum", bufs=2, space="PSUM") as upsum,
            tc.tile_pool(name="bconst", bufs=1) as bconst,
        ):
            htab = dram.tile([cfg.Np, OUT], F16)
            ptabs = []
            rsouts = []
            for k in range(PIECES):
                pt = dram.tile([C, P, cfg.piece_nw[k] * P], F16, name=f"ptab{k}")
                ro = dram.tile([P, cfg.piece_nw[k] * P], F16, name=f"rsout{k}")
                ptabs.append(pt)
                rsouts.append(ro)
            nc.gpsimd.load_library(library_config.mlp)

            # ---- phase A: down-projection into the local DRAM h table ----
            with (
                tc.tile_pool(name="aconst", bufs=1) as aconst,
                tc.tile_pool(name="xat", bufs=3) as xap,
                tc.tile_pool(name="hst", bufs=1) as hstp,
            ):
                wd_sb = aconst.tile([P, cfg.KIN * OUT], F16)
                nc.sync.dma_start(out=wd_sb[:], in_=wd[:, :])
                h_stage = hstp.tile([P, W * OUT], F16)
                xa_v = xa[:, :].rearrange("p (w x) -> p w x", w=W)
                qbounds = [0]
                step0 = max(4, W // 12)
                qbounds.append(min(step0, W))
                while qbounds[-1] < W:
                    qbounds.append(min(qbounds[-1] + (W + 3) // 4, W))
                for qd in range(len(qbounds) - 1):
                    lo = qbounds[qd]
                    hi = qbounds[qd + 1]
                    xt = xap.tile([P, (hi - lo) * cfg.KIN * P], F16)
                    xa_eng = nc.sync if qd % 2 == 0 else nc.scalar
                    xa_eng.dma_start(
                        out=xt[:].rearrange("p (w x) -> p w x", w=hi - lo),
                        in_=xa_v[:, lo:hi, :],
                    )
                    for w in range(lo, hi):
                        blkb = (w - lo) * cfg.KIN * P
                        psfull = gpsum.tile([P, (WPG + 1) * P], F32, space="PSUM", tag="PS")
                        ps = psfull[:, 0:OUT]
                        for ccx in range(cfg.KC):
                            nc.tensor.matmul(
                                ps[:],
                                lhsT=xt[:, blkb + ccx * P:blkb + (ccx + 1) * P],
                                rhs=wd_sb[:, ccx * OUT:(ccx + 1) * OUT],
                                start=(ccx == 0),
                                stop=False,
                            )
                        nc.tensor.matmul(
                            ps[:],
                            lhsT=xt[0:1, blkb + cfg.KC * P:blkb + (cfg.KC + 1) * P],
                            rhs=wd_sb[0:1, cfg.KC * OUT:cfg.KC * OUT + OUT],
                            start=False,
                            stop=True,
                        )
                        nc.vector.tensor_copy(
                            out=h_stage[:, w * OUT:(w + 1) * OUT], in_=ps[:]
                        )
                    nc.scalar.dma_start(
                        out=htab[lo * P:hi * P, :].rearrange("(w p) f -> p w f", p=P),
                        in_=h_stage[:, lo * OUT:hi * OUT].rearrange(
                            "p (w f) -> p w f", w=hi - lo
                        ),
                    )

            # phase-B constants: issued after the xa loads so the first
            # gather's idx data rides the DMA rails during the down-proj tail
            wgu_sb = bconst.tile([OUT, IN], F16)
            nc.sync.dma_start(out=wgu_sb[:], in_=wgu[:, :])
            cst = bconst.tile([P, IOTA_W], F16)
            nc.sync.dma_start(out=cst[:], in_=consts[:, :])
            ndst_sb = bconst.tile([P, W], F32)
            nc.sync.dma_start(out=ndst_sb[:], in_=ndstw[:, :])
            idx_sb = bconst.tile([P, S16], I16)
            nc.scalar.dma_start(out=idx_sb[:], in_=idx[:, :])
            slot_sb = bconst.tile([P, NBLK], F32)
            nc.scalar.dma_start(out=slot_sb[:], in_=slotv[:, :])

            # ---- phase B: src-side aggregation, piece-wise RS, up-projection ----
            with (
                tc.tile_pool(name="gp", bufs=3) as gp,
                tc.tile_pool(name="sp", bufs=12) as sp,
                tc.tile_pool(name="stg", bufs=4) as stgp,
                tc.tile_pool(name="agr", bufs=2) as agrp,
                tc.tile_pool(name="osb", bufs=2) as osb,
            ):
                cell_tiles = {}

                def emit_gathers(k):
                    for ci in piece_cells[k]:
                        o16, L = gathers[ci]
                        gt = gp.tile([P, (L // 128) * OUT], F16, tag="G", name=f"g{ci}")
                        nc.gpsimd.dma_gather(
                            out_ap=gt[:].rearrange("p (b f) -> p b f", b=L // 128),
                            in_ap=htab[:, :],
                            idxs_ap=idx_sb[:, o16:o16 + L // 16],
                            num_idxs=L,
                            num_idxs_reg=L,
                            elem_size=OUT,
                            single_packet=False,
                        )
                        cell_tiles[ci] = gt

                def emit_agg(k):
                    for s in range(sg_base[k], sg_base[k + 1]):
                        ci, b0 = gather_of_sg[s]
                        gt = cell_tiles[ci]
                        c = (s - sg_base[k]) // PIECE_G[k]
                        gi = (s - sg_base[k]) % PIECE_G[k]
                        psg = gpsum.tile([P, (WPG + 1) * P], F32, space="PSUM", tag="PS")
                        for s2, kb, lo, nh, bi in blocks_by_sg[s]:
                            S = sp.tile([P, nh * P], F16, tag="S")
                            nc.vector.tensor_scalar(
                                out=S[:],
                                in0=cst[:, 0:nh * P],
                                scalar1=slot_sb[:, bi:bi + 1],
                                scalar2=None,
                                op0=mybir.AluOpType.is_equal,
                            )
                            for h in range(nh):
                                ws = lo + h
                                nc.tensor.matmul(
                                    psg[:, ws * P:(ws + 1) * P],
                                    lhsT=gt[:, (bi - b0) * OUT:(bi - b0 + 1) * OUT],
                                    rhs=S[:, h * P:(h + 1) * P],
                                    start=(first_mm.get((s, ws // 4)) == (bi, h)),
                                    stop=(last_mm.get((s, ws // 4)) == (bi, h)),
                                )
                        stg = stgp.tile([P, WPG * P], F16, tag="D")
                        nc.scalar.activation(
                            out=stg[:],
                            in_=psg[:, 0:WPG * P],
                            func=mybir.ActivationFunctionType.Copy,
                        )
                        nc.sync.dma_start(
                            out=ptabs[k][c, :, gi * WPG * P:(gi + 1) * WPG * P],
                            in_=stg[:],
                        )

                def emit_rs(k):
                    # on the Pool queue (walrus requires Pool for collectives);
                    # issued AFTER piece k+1's gathers so it never stalls them
                    nc.gpsimd.collective_compute(
                        "ReduceScatter",
                        mybir.AluOpType.add,
                        replica_groups=[list(range(C))],
                        ins=[ptabs[k][:].opt()],
                        outs=[rsouts[k][:].opt()],
                    )

                def emit_upproj(k):
                    nw = cfg.piece_nw[k]
                    aggR = agrp.tile([P, nw * P], F16, tag="A", name=f"ar{k}")
                    nc.sync.dma_start(out=aggR[:], in_=rsouts[k][:, :])
                    for gi2 in range(PIECE_G[k]):
                        ost = osb.tile([P, WPG * IN], F16, tag="O")
                        for wl in range(WPG):
                            w = cfg.piece_w0[k] + gi2 * WPG + wl
                            ps2 = upsum.tile([P, IN], F32, space="PSUM", tag="UP")
                            for lo2 in range(0, IN, 512):
                                hi2 = min(lo2 + 512, IN)
                                nc.tensor.matmul(
                                    ps2[:, lo2:hi2],
                                    lhsT=aggR[:, (gi2 * WPG + wl) * P:(gi2 * WPG + wl + 1) * P],
                                    rhs=wgu_sb[:, lo2:hi2],
                                    start=True,
                                    stop=True,
                                )
                            if k != 1:
                                nc.scalar.activation(
                                    out=ost[:, wl * IN:(wl + 1) * IN],
                                    in_=ps2[:],
                                    func=mybir.ActivationFunctionType.Copy,
                                    scale=ndst_sb[:, w:w + 1],
                                )
                            else:
                                nc.vector.tensor_scalar(
                                    out=ost[:, wl * IN:(wl + 1) * IN],
                                    in0=ps2[:],
                                    scalar1=ndst_sb[:, w:w + 1],
                                    scalar2=None,
                                    op0=mybir.AluOpType.mult,
                                )
                        w0 = cfg.piece_w0[k] + gi2 * WPG
                        nc.sync.dma_start(
                            out=out[w0 * P:(w0 + WPG) * P, :].rearrange(
                                "(w p) f -> p w f", p=P
                            ),
                            in_=ost[:].rearrange("p (w f) -> p w f", w=WPG),
                        )

                for k in range(PIECES):
                    emit_gathers(k)
                for k in range(PIECES):
                    emit_agg(k)
                    emit_rs(k)
                    if k >= 1:
                        emit_upproj(k - 1)
                emit_upproj(PIECES - 1)

    nc.compile()
    return nc


_GRAPH_CACHE = {}


def kernel(features, Wd, bd, Wg, bg, Wu, bu, src, dst):
    features = np.asarray(features)
    N, IN = features.shape
    OUT = np.asarray(Wd).shape[1]
    E = np.asarray(src).shape[0]
    cfg = Cfg(N, E, IN, OUT)

    in_maps, node_core, node_slot, prof = host_prep(
        cfg, features, Wd, bd, Wg, bg, Wu, bu, src, dst
    )
    key = (N, E, IN, OUT, prof["T"], prof["blocks"])
    nc = _GRAPH_CACHE.get(key)
    if nc is None:
        nc = build_graph(cfg, prof)
        _GRAPH_CACHE[key] = nc

    res = run_bass_kernel_spmd(nc, in_maps, core_ids=list(range(cfg.C)))
    allo = np.stack([np.asarray(res.results[i]["out"]) for i in range(cfg.C)])
    out = allo[node_core, node_slot, :].astype(np.float32)
    bu2 = (np.asarray(bg, np.float64) @ np.asarray(Wu, np.float64) + bu).astype(
        np.float32
    )
    out += bu2[None, :]
    return out


# revision 15
# speedup vs baseline: 1.0185x; 1.0185x over previous
"""AdapterGNN distributed Trainium2 kernel (8 NeuronCores, Bass/Tile).

out = norm_dst * segsum_dst( ((X*norm_src) @ Wd + norm_src*bd)[src] ) @ (Wg@Wu) + (bg@Wu+bu)

Sharding (src-side aggregation + ReduceScatter):
  Nodes are split contiguously across 8 cores. Each core down-projects its own
  node block (h, fp16) into a LOCAL DRAM table, then processes the edges whose
  SRC it owns: per-edge h rows are fetched with batched dma_gather instructions
  from the local table (no pre-collective!), and a PE segment-sum accumulates
  per-dst-window partial aggregates: for each 128-edge block, a selection
  matrix S[e, d] = is_equal(iota, slot_e) is built on the DVE and
  psum[f, d] += G_blk^T-contract-S.  Partials are drained FEATURE-major
  ([128 f, dst]) so the DRAM writes are >=1792B runs (full DMA bandwidth).

  The global dst space (8 chunks x 98 windows) is processed in 7 "pieces"
  (14 windows of every chunk per piece); after each piece a ReduceScatter(add)
  combines the 8 cores' partials and hands each core its own dst rows --
  7 small collectives (~26.5us each) pipelined behind the gather stream
  instead of one 284us AllGather blocking everything.

  Per-piece tail: the RS output (aggT, f-major = exactly the up-projection's
  lhsT layout) is loaded back, multiplied by the fused (Wg@Wu) weight, and the
  Activation-engine psum drain applies norm_dst as its per-partition scale.
  norm_src and bd ride in the down-projection inputs (host-folded); bg@Wu+bu
  is added on the host.

Self-contained: requires only numpy + concourse (+ TRN2 cores via axon).
"""

import numpy as np

import concourse.bacc as bacc
import concourse.bass as bass
import concourse.mybir as mybir
import concourse.tile as tile
from concourse import library_config
from concourse.bass_utils import run_bass_kernel_spmd

F32 = mybir.dt.float32
F16 = mybir.dt.float16
I16 = mybir.dt.int16

P = 128           # partitions
WPG = 7           # windows per psum group ((WPG+1)*128 f32 = 2 psum banks)
PIECE_G = (7, 6, 1)   # groups-of-7 per (piece, chunk): 49+42+7 = 98 windows
MAXH = 4          # max windows one 128-edge block may span
IOTA_W = MAXH * P
GATHER_MAX = 10240    # target idxs per gather instruction


class Cfg:
    def __init__(self, n_nodes, n_edges, in_dim, out_dim, n_cores=8):
        self.N = n_nodes
        self.E = n_edges
        self.IN = in_dim          # 768
        self.OUT = out_dim        # 128 (must be 128)
        self.C = n_cores
        assert out_dim == P
        self.NpReal = (n_nodes + n_cores - 1) // n_cores   # real nodes per core
        self.W = (self.NpReal + P - 1) // P                # windows per chunk
        assert self.W == sum(PIECE_G) * WPG, self.W
        self.PIECES = len(PIECE_G)
        self.Np = self.W * P                               # padded nodes/chunk
        self.KC = in_dim // P                              # full K chunks (6)
        assert in_dim % P == 0
        self.KIN = self.KC + 1                             # +1 chunk: (norm,bias) row
        self.GPC = sum(PIECE_G)                            # groups per chunk (14)
        self.NSG = n_cores * self.GPC                      # supergroups (112)
        # window offset of each piece within a chunk, in windows
        self.piece_w0 = [WPG * sum(PIECE_G[:k]) for k in range(self.PIECES)]
        self.piece_nw = [WPG * g for g in PIECE_G]


def _ceil128(x):
    return int(-(-int(x) // 128) * 128)


def host_prep(cfg, features, Wd, bd, Wg, bg, Wu, bu, src, dst):
    """Returns (in_maps, node_core, node_slot, prof)."""
    C, N, Np, W = cfg.C, cfg.N, cfg.Np, cfg.W
    src = np.asarray(src).astype(np.int64)
    dst = np.asarray(dst).astype(np.int64)
    features = np.asarray(features, dtype=np.float32)

    out_deg = np.bincount(src, minlength=N)
    in_deg = np.bincount(dst, minlength=N)
    norm_src = 1.0 / np.sqrt(np.maximum(out_deg, 1.0))
    norm_dst = 1.0 / np.sqrt(np.maximum(in_deg, 1.0))

    nodes = np.arange(N)
    node_core = np.minimum(nodes // cfg.NpReal, C - 1)
    node_slot = nodes - node_core * cfg.NpReal

    # per-edge decomposition; edge owned by src's core
    ecore = node_core[src]
    erow = node_slot[src]                      # local table row of h[src]
    dchunk = node_core[dst]
    dloc = node_slot[dst]
    dwc = dloc // P                            # window within dst chunk (0..97)
    dp = dloc % P
    gc = dwc // WPG                            # group within chunk (0..13)
    wig = dwc % WPG                            # window in group (0..6)
    # piece of each group-of-7 and group index within the piece
    gcum = np.cumsum((0,) + PIECE_G)           # [0, 7, 13, 14]
    kp_of_g = np.searchsorted(gcum[1:], np.arange(cfg.GPC), side="right")
    gip_of_g = np.arange(cfg.GPC) - gcum[kp_of_g]
    kp = kp_of_g[gc]                           # piece (0..2)
    gi = gip_of_g[gc]                          # group in (piece, chunk)
    # supergroup = program-order group index: by (piece, chunk, group)
    sg_base = np.concatenate([[0], np.cumsum([PIECE_G[k] * C for k in range(cfg.PIECES)])])
    sg = sg_base[kp] + dchunk * np.asarray(PIECE_G)[kp] + gi
    NSG = cfg.NSG

    # shared per-(sg, window) maxima
    NC = np.zeros((C, NSG, WPG), dtype=np.int64)
    np.add.at(NC, (ecore, sg, wig), 1)
    M = NC.max(axis=0)                         # [NSG, WPG]

    # blocks per supergroup chunk (stream ceil128 per sg)
    chunk_off = np.zeros(NSG, dtype=np.int64)
    chunk_len = np.zeros(NSG, dtype=np.int64)
    blocks = []       # (sg, k_in_chunk, lo, nh, bi)
    off = 0
    for s in range(NSG):
        seg = M[s]
        L = _ceil128(seg.sum())
        chunk_off[s] = off
        chunk_len[s] = L
        bcum = np.concatenate([[0], np.cumsum(seg)])
        for k in range(L // 128):
            p0, p1 = k * 128, k * 128 + 127
            lo = int(np.searchsorted(bcum[1:], p0, side="right"))
            hi = int(np.searchsorted(bcum[1:], p1, side="right"))
            lo, hi = min(lo, WPG - 1), min(hi, WPG - 1)
            nh = hi - lo + 1
            assert nh <= MAXH, f"block spans {nh} windows"
            blocks.append((s, k, lo, nh, len(blocks)))
        off += L
    T = int(off)
    NBLK = len(blocks)
    assert T == NBLK * 128

    # psum zero-region start/stop flags per (sg, 2KB region = ws//4)
    first_mm = {}
    last_mm = {}
    for s, k, lo, nh, bi in blocks:
        for h in range(nh):
            key2 = (s, (lo + h) // 4)
            if key2 not in first_mm:
                first_mm[key2] = (bi, h)
            last_mm[key2] = (bi, h)

    # gather instruction cells: greedy-pack consecutive sgs (never across pieces)
    sg_base_l = [0]
    for k in range(cfg.PIECES):
        sg_base_l.append(sg_base_l[-1] + PIECE_G[k] * C)
    gathers = []          # (o16, L)
    gather_of_sg = {}     # sg -> (cell idx, o16*16//128 = first block idx)
    piece_cells = []      # per piece: list of cell indices
    for k in range(cfg.PIECES):
        cells = []
        s0 = sg_base_l[k]
        send = sg_base_l[k + 1]
        cur = s0
        while cur < send:
            o = int(chunk_off[cur])
            L = 0
            first = cur
            while cur < send and (L == 0 or L + chunk_len[cur] <= GATHER_MAX):
                L += int(chunk_len[cur])
                cur += 1
            ci = len(gathers)
            gathers.append((o // 16, L))
            for s2 in range(first, cur):
                gather_of_sg[s2] = (ci, o // 128)
            cells.append(ci)
        piece_cells.append(tuple(cells))

    prof = {
        "chunk_off": tuple(int(x) for x in chunk_off),
        "chunk_len": tuple(int(x) for x in chunk_len),
        "blocks": tuple(blocks),
        "first": frozenset(first_mm.items()),
        "last": frozenset(last_mm.items()),
        "gathers": tuple(gathers),
        "gather_of_sg": tuple(sorted(gather_of_sg.items())),
        "piece_cells": tuple(tuple(x) for x in piece_cells),
        "sg_base": tuple(sg_base_l),
        "T": T,
        "NBLK": NBLK,
    }

    # fused weights
    Wgu = (np.asarray(Wg, np.float64) @ np.asarray(Wu, np.float64)).astype(np.float32)
    bu2 = (np.asarray(bg, np.float64) @ np.asarray(Wu, np.float64) + bu).astype(np.float32)

    wd_h = np.zeros((P, cfg.KIN * cfg.OUT), dtype=np.float16)
    for cc in range(cfg.KC):
        wd_h[:, cc * cfg.OUT:(cc + 1) * cfg.OUT] = Wd[cc * P:(cc + 1) * P, :]
    wd_h[0, cfg.KC * cfg.OUT:(cfg.KC + 1) * cfg.OUT] = bd
    wgu_h = Wgu.astype(np.float16)

    consts = np.zeros((P, IOTA_W), dtype=np.float16)
    consts[:, :] = np.arange(IOTA_W, dtype=np.float16)[None, :]

    # per-block lo for slot_rel
    blk_lo = np.zeros(NBLK, dtype=np.int64)
    for s, k, lo, nh, bi in blocks:
        assert chunk_off[s] // 128 + k == bi
        blk_lo[bi] = lo

    # intra-chunk window segment offsets (shared)
    segoff = np.zeros((NSG, WPG), dtype=np.int64)
    for s in range(NSG):
        segoff[s] = chunk_off[s] + np.concatenate([[0], np.cumsum(M[s])[:-1]])

    S16 = T // 16
    in_maps = []
    for c in range(C):
        em = np.where(ecore == c)[0]
        sgc, wigc, dpc = sg[em], wig[em], dp[em]
        order = np.lexsort((dpc, wigc, sgc))
        em, sgc, wigc, dpc = em[order], sgc[order], wigc[order], dpc[order]
        segid = sgc * WPG + wigc
        uniq, counts = np.unique(segid, return_counts=True)
        cum = np.concatenate([[0], np.cumsum(counts)])
        rank = np.arange(len(em)) - cum[np.searchsorted(uniq, segid)]
        pos = segoff[sgc, wigc] + rank
        assert (rank < M[sgc, wigc]).all()

        idx_s = np.zeros(T, dtype=np.int64)
        slotg = np.full(T, -1, dtype=np.int64)   # slot within group (wig*128+p)
        idx_s[pos] = erow[em]
        slotg[pos] = wigc * P + dpc

        # per-block relative slots
        slot_rel = slotg.reshape(-1, 128) - blk_lo[:, None] * P
        slot_rel[slotg.reshape(-1, 128) < 0] = -1
        assert (slot_rel < MAXH * P).all()

        idx16 = np.zeros((P, S16), dtype=np.int16)
        sidx = np.arange(S16) * 16
        for p in range(P):
            idx16[p, :] = idx_s[sidx + (p % 16)]
        slotv = np.ascontiguousarray(slot_rel.T.astype(np.float32))   # [128, NBLK]

        # xa: window-blocked [p, w*KIN*128 + cc*128 + n]; own nodes, norm_src folded
        nt_ids = np.where(node_core == np.int64(c))[0]
        xs = (features[nt_ids, :] * norm_src[nt_ids, None]).astype(np.float16)
        xa = np.zeros((P, W * cfg.KIN * P), dtype=np.float16)
        xs_slot = np.zeros((Np, cfg.IN), dtype=np.float16)
        xs_slot[node_slot[nt_ids], :] = xs
        nsr = np.zeros(Np, dtype=np.float16)
        nsr[node_slot[nt_ids]] = norm_src[nt_ids].astype(np.float16)
        for w in range(W):
            blkb = w * cfg.KIN * P
            rows = xs_slot[w * P:(w + 1) * P, :]
            for cc in range(cfg.KC):
                xa[:, blkb + cc * P:blkb + (cc + 1) * P] = rows[:, cc * P:(cc + 1) * P].T
            xa[0, blkb + cfg.KC * P:blkb + (cfg.KC + 1) * P] = nsr[w * P:(w + 1) * P]

        # own-chunk norm_dst per (partition, window)
        ndstw = np.zeros((P, W), dtype=np.float32)
        nd = np.zeros(Np, dtype=np.float32)
        nd[node_slot[nt_ids]] = norm_dst[nt_ids].astype(np.float32)
        ndstw[:, :] = nd.reshape(W, P).T

        in_maps.append(
            {
                "xa": xa,
                "idx": idx16,
                "slotv": slotv,
                "wd": wd_h,
                "wgu": wgu_h,
                "consts": consts,
                "ndstw": ndstw,
            }
        )

    return in_maps, node_core, node_slot, prof


def build_graph(cfg, prof):
    """Build the SPMD Bass graph (same for all cores)."""
    W, OUT, IN = cfg.W, cfg.OUT, cfg.IN
    C, PIECES = cfg.C, cfg.PIECES
    blocks = prof["blocks"]
    first_mm = dict(prof["first"])
    last_mm = dict(prof["last"])
    gathers = prof["gathers"]
    gather_of_sg = dict(prof["gather_of_sg"])
    piece_cells = prof["piece_cells"]
    sg_base = prof["sg_base"]
    T = prof["T"]
    NBLK = prof["NBLK"]
    S16 = T // 16

    blocks_by_sg = {}
    for b in blocks:
        blocks_by_sg.setdefault(b[0], []).append(b)

    nc = bacc.Bacc(None, target_bir_lowering=False)
    xa = nc.declare_dram_parameter("xa", [P, W * cfg.KIN * P], F16, False)
    idx = nc.declare_dram_parameter("idx", [P, S16], I16, False)
    slotv = nc.declare_dram_parameter("slotv", [P, NBLK], F32, False)
    wd = nc.declare_dram_parameter("wd", [P, cfg.KIN * OUT], F16, False)
    wgu = nc.declare_dram_parameter("wgu", [OUT, IN], F16, False)
    consts = nc.declare_dram_parameter("consts", [P, IOTA_W], F16, False)
    ndstw = nc.declare_dram_parameter("ndstw", [P, W], F32, False)
    out = nc.declare_dram_parameter("out", [cfg.Np, IN], F16, True)

    with tile.TileContext(nc) as tc:
        with (
            tc.tile_pool(name="dram", bufs=1, space="DRAM") as dram,
            tc.tile_pool(name="gpsum", bufs=2, space="PSUM") as gpsum,
            tc.tile_pool(name="upsum", bufs=2, space="PSUM") as upsum,
            tc.tile_pool(name="bconst", bufs=1) as bconst,
        ):
            htab = dram.tile([cfg.Np, OUT], F16)
            ptabs = []
            rsouts = []
            for k in range(PIECES):
                pt = dram.tile([C, P, cfg.piece_nw[k] * P], F16, name=f"ptab{k}")
                ro = dram.tile([P, cfg.piece_nw[k] * P], F16, name=f"rsout{k}")
                ptabs.append(pt)
                rsouts.append(ro)
            nc.gpsimd.load_library(library_config.mlp)

            # ---- phase A: down-projection into the local DRAM h table ----
            with (
                tc.tile_pool(name="aconst", bufs=1) as aconst,
                tc.tile_pool(name="xat", bufs=3) as xap,
                tc.tile_pool(name="hst", bufs=1) as hstp,
            ):
                wd_sb = aconst.tile([P, cfg.KIN * OUT], F16)
                nc.sync.dma_start(out=wd_sb[:], in_=wd[:, :])
                h_stage = hstp.tile([P, W * OUT], F16)
                xa_v = xa[:, :].rearrange("p (w x) -> p w x", w=W)
                qbounds = [0]
                step0 = max(4, W // 12)
                qbounds.append(min(step0, W))
                while qbounds[-1] < W:
                    qbounds.append(min(qbounds[-1] + (W + 3) // 4, W))
                for qd in range(len(qbounds) - 1):
                    lo = qbounds[qd]
                    hi = qbounds[qd + 1]
                    xt = xap.tile([P, (hi - lo) * cfg.KIN * P], F16)
                    xa_eng = nc.sync if qd % 2 == 0 else nc.scalar
                    xa_eng.dma_start(
                        out=xt[:].rearrange("p (w x) -> p w x", w=hi - lo),
                        in_=xa_v[:, lo:hi, :],
                    )
                    for w in range(lo, hi):
                        blkb = (w - lo) * cfg.KIN * P
                        psfull = gpsum.tile([P, (WPG + 1) * P], F32, space="PSUM", tag="PS")
                        ps = psfull[:, 0:OUT]
                        for ccx in range(cfg.KC):
                            nc.tensor.matmul(
                                ps[:],
                                lhsT=xt[:, blkb + ccx * P:blkb + (ccx + 1) * P],
                                rhs=wd_sb[:, ccx * OUT:(ccx + 1) * OUT],
                                start=(ccx == 0),
                                stop=False,
                            )
                        nc.tensor.matmul(
                            ps[:],
                            lhsT=xt[0:1, blkb + cfg.KC * P:blkb + (cfg.KC + 1) * P],
                            rhs=wd_sb[0:1, cfg.KC * OUT:cfg.KC * OUT + OUT],
                            start=False,
                            stop=True,
                        )
                        nc.vector.tensor_copy(
                            out=h_stage[:, w * OUT:(w + 1) * OUT], in_=ps[:]
                        )
                    nc.scalar.dma_start(
                        out=htab[lo * P:hi * P, :].rearrange("(w p) f -> p w f", p=P),
                        in_=h_stage[:, lo * OUT:hi * OUT].rearrange(
                            "p (w f) -> p w f", w=hi - lo
                        ),
                    )

            # phase-B constants: issued after the xa loads so the first
            # gather's idx data rides the DMA rails during the down-proj tail
            wgu_sb = bconst.tile([OUT, IN], F16)
            nc.sync.dma_start(out=wgu_sb[:], in_=wgu[:, :])
            cst = bconst.tile([P, IOTA_W], F16)
            nc.sync.dma_start(out=cst[:], in_=consts[:, :])
            ndst_sb = bconst.tile([P, W], F32)
            nc.sync.dma_start(out=ndst_sb[:], in_=ndstw[:, :])
            idx_sb = bconst.tile([P, S16], I16)
            nc.scalar.dma_start(out=idx_sb[:], in_=idx[:, :])
            slot_sb = bconst.tile([P, NBLK], F32)
            nc.scalar.dma_start(out=slot_sb[:], in_=slotv[:, :])

            # ---- phase B: src-side aggregation, piece-wise RS, up-projection ----
            with (
                tc.tile_pool(name="gp", bufs=3) as gp,
                tc.tile_pool(name="sp", bufs=12) as sp,
                tc.tile_pool(name="stg", bufs=4) as stgp,
                tc.tile_pool(name="agr", bufs=2) as agrp,
                tc.tile_pool(name="osb", bufs=2) as osb,
            ):
                cell_tiles = {}

                def emit_gathers(k):
                    for ci in piece_cells[k]:
                        o16, L = gathers[ci]
                        gt = gp.tile([P, (L // 128) * OUT], F16, tag="G", name=f"g{ci}")
                        nc.gpsimd.dma_gather(
                            out_ap=gt[:].rearrange("p (b f) -> p b f", b=L // 128),
                            in_ap=htab[:, :],
                            idxs_ap=idx_sb[:, o16:o16 + L // 16],
                            num_idxs=L,
                            num_idxs_reg=L,
                            elem_size=OUT,
                            single_packet=False,
                        )
                        cell_tiles[ci] = gt

                def emit_agg(k):
                    for s in range(sg_base[k], sg_base[k + 1]):
                        ci, b0 = gather_of_sg[s]
                        gt = cell_tiles[ci]
                        c = (s - sg_base[k]) // PIECE_G[k]
                        gi = (s - sg_base[k]) % PIECE_G[k]
                        psg = gpsum.tile([P, (WPG + 1) * P], F32, space="PSUM", tag="PS")
                        for s2, kb, lo, nh, bi in blocks_by_sg[s]:
                            S = sp.tile([P, nh * P], F16, tag="S")
                            nc.vector.tensor_scalar(
                                out=S[:],
                                in0=cst[:, 0:nh * P],
                                scalar1=slot_sb[:, bi:bi + 1],
                                scalar2=None,
                                op0=mybir.AluOpType.is_equal,
                            )
                            for h in range(nh):
                                ws = lo + h
                                nc.tensor.matmul(
                                    psg[:, ws * P:(ws + 1) * P],
                                    lhsT=gt[:, (bi - b0) * OUT:(bi - b0 + 1) * OUT],
                                    rhs=S[:, h * P:(h + 1) * P],
                                    start=(first_mm.get((s, ws // 4)) == (bi, h)),
                                    stop=(last_mm.get((s, ws // 4)) == (bi, h)),
                                )
                        stg = stgp.tile([P, WPG * P], F16, tag="D")
                        nc.scalar.activation(
                            out=stg[:],
                            in_=psg[:, 0:WPG * P],
                            func=mybir.ActivationFunctionType.Copy,
                        )
                        nc.sync.dma_start(
                            out=ptabs[k][c, :, gi * WPG * P:(gi + 1) * WPG * P],
                            in_=stg[:],
                        )

                def emit_rs(k):
                    # on the Pool queue (walrus requires Pool for collectives);
                    # issued AFTER piece k+1's gathers so it never stalls them
                    nc.gpsimd.collective_compute(
                        "ReduceScatter",
                        mybir.AluOpType.add,
                        replica_groups=[list(range(C))],
                        ins=[ptabs[k][:].opt()],
                        outs=[rsouts[k][:].opt()],
                    )

                def emit_upproj(k):
                    nw = cfg.piece_nw[k]
                    ld_eng = nc.scalar if k % 2 == 0 else nc.sync
                    aggR = agrp.tile([P, nw * P], F16, tag="A", name=f"ar{k}")
                    ld_eng.dma_start(out=aggR[:], in_=rsouts[k][:, :])
                    for gi2 in range(PIECE_G[k]):
                        ost = osb.tile([P, WPG * IN], F16, tag="O")
                        for wl in range(WPG):
                            w = cfg.piece_w0[k] + gi2 * WPG + wl
                            ps2 = upsum.tile([P, IN], F32, space="PSUM", tag="UP")
                            for lo2 in range(0, IN, 512):
                                hi2 = min(lo2 + 512, IN)
                                nc.tensor.matmul(
                                    ps2[:, lo2:hi2],
                                    lhsT=aggR[:, (gi2 * WPG + wl) * P:(gi2 * WPG + wl + 1) * P],
                                    rhs=wgu_sb[:, lo2:hi2],
                                    start=True,
                                    stop=True,
                                )
                            if k != 1:
                                nc.scalar.activation(
                                    out=ost[:, wl * IN:(wl + 1) * IN],
                                    in_=ps2[:],
                                    func=mybir.ActivationFunctionType.Copy,
                                    scale=ndst_sb[:, w:w + 1],
                                )
                            else:
                                nc.vector.tensor_scalar(
                                    out=ost[:, wl * IN:(wl + 1) * IN],
                                    in0=ps2[:],
                                    scalar1=ndst_sb[:, w:w + 1],
                                    scalar2=None,
                                    op0=mybir.AluOpType.mult,
                                )
                        w0 = cfg.piece_w0[k] + gi2 * WPG
                        out_eng = nc.scalar if k % 2 == 0 else nc.sync
                        out_eng.dma_start(
                            out=out[w0 * P:(w0 + WPG) * P, :].rearrange(
                                "(w p) f -> p w f", p=P
                            ),
                            in_=ost[:].rearrange("p (w f) -> p w f", w=WPG),
                        )

                for k in range(PIECES):
                    emit_gathers(k)
                for k in range(PIECES):
                    emit_agg(k)
                    emit_rs(k)
                    if k >= 1:
                        emit_upproj(k - 1)
                emit_upproj(PIECES - 1)

    nc.compile()
    return nc


_GRAPH_CACHE = {}


def kernel(features, Wd, bd, Wg, bg, Wu, bu, src, dst):
    features = np.asarray(features)
    N, IN = features.shape
    OUT = np.asarray(Wd).shape[1]
    E = np.asarray(src).shape[0]
    cfg = Cfg(N, E, IN, OUT)

    in_maps, node_core, node_slot, prof = host_prep(
        cfg, features, Wd, bd, Wg, bg, Wu, bu, src, dst
    )
    key = (N, E, IN, OUT, prof["T"], prof["blocks"])
    nc = _GRAPH_CACHE.get(key)
    if nc is None:
        nc = build_graph(cfg, prof)
        _GRAPH_CACHE[key] = nc

    res = run_bass_kernel_spmd(nc, in_maps, core_ids=list(range(cfg.C)))
    allo = np.stack([np.asarray(res.results[i]["out"]) for i in range(cfg.C)])
    out = allo[node_core, node_slot, :].astype(np.float32)
    bu2 = (np.asarray(bg, np.float64) @ np.asarray(Wu, np.float64) + bu).astype(
        np.float32
    )
    out += bu2[None, :]
    return out


# revision 17
# speedup vs baseline: 1.0520x; 1.0329x over previous
"""AdapterGNN distributed Trainium2 kernel (8 NeuronCores, Bass/Tile).

out = norm_dst * segsum_dst( ((X*norm_src) @ Wd + norm_src*bd)[src] ) @ (Wg@Wu) + (bg@Wu+bu)

Sharding (src-side aggregation + ReduceScatter):
  Nodes are split contiguously across 8 cores. Each core down-projects its own
  node block (h, fp16) into a LOCAL DRAM table, then processes the edges whose
  SRC it owns: per-edge h rows are fetched with batched dma_gather instructions
  from the local table (no pre-collective!), and a PE segment-sum accumulates
  per-dst-window partial aggregates: for each 128-edge block, a selection
  matrix S[e, d] = is_equal(iota, slot_e) is built on the DVE and
  psum[f, d] += G_blk^T-contract-S.  Partials are drained FEATURE-major
  ([128 f, dst]) so the DRAM writes are >=1792B runs (full DMA bandwidth).

  The global dst space (8 chunks x 98 windows) is processed in 7 "pieces"
  (14 windows of every chunk per piece); after each piece a ReduceScatter(add)
  combines the 8 cores' partials and hands each core its own dst rows --
  7 small collectives (~26.5us each) pipelined behind the gather stream
  instead of one 284us AllGather blocking everything.

  Per-piece tail: the RS output (aggT, f-major = exactly the up-projection's
  lhsT layout) is loaded back, multiplied by the fused (Wg@Wu) weight, and the
  Activation-engine psum drain applies norm_dst as its per-partition scale.
  norm_src and bd ride in the down-projection inputs (host-folded); bg@Wu+bu
  is added on the host.

Self-contained: requires only numpy + concourse (+ TRN2 cores via axon).
"""

import numpy as np

import concourse.bacc as bacc
import concourse.bass as bass
import concourse.mybir as mybir
import concourse.tile as tile
from concourse import library_config
from concourse.bass_utils import run_bass_kernel_spmd

F32 = mybir.dt.float32
F16 = mybir.dt.float16
I16 = mybir.dt.int16

P = 128           # partitions
WPG = 7           # windows per psum group ((WPG+1)*128 f32 = 2 psum banks)
PIECE_G = (7, 6, 1)   # groups-of-7 per (piece, chunk): 49+42+7 = 98 windows
MAXH = 4          # max windows one 128-edge block may span
IOTA_W = MAXH * P
GATHER_MAX = 10240    # target idxs per gather instruction


class Cfg:
    def __init__(self, n_nodes, n_edges, in_dim, out_dim, n_cores=8):
        self.N = n_nodes
        self.E = n_edges
        self.IN = in_dim          # 768
        self.OUT = out_dim        # 128 (must be 128)
        self.C = n_cores
        assert out_dim == P
        self.NpReal = (n_nodes + n_cores - 1) // n_cores   # real nodes per core
        self.W = (self.NpReal + P - 1) // P                # windows per chunk
        assert self.W == sum(PIECE_G) * WPG, self.W
        self.PIECES = len(PIECE_G)
        self.Np = self.W * P                               # padded nodes/chunk
        self.KC = in_dim // P                              # full K chunks (6)
        assert in_dim % P == 0
        self.KIN = self.KC + 1                             # +1 chunk: (norm,bias) row
        self.GPC = sum(PIECE_G)                            # groups per chunk (14)
        self.NSG = n_cores * self.GPC                      # supergroups (112)
        # window offset of each piece within a chunk, in windows
        self.piece_w0 = [WPG * sum(PIECE_G[:k]) for k in range(self.PIECES)]
        self.piece_nw = [WPG * g for g in PIECE_G]


def _ceil128(x):
    return int(-(-int(x) // 128) * 128)



def _balance_src(cfg, src, dst, node_core, node_slot):
    """Greedy src->core assignment balancing per-(global window) edge counts."""
    C, N, W = cfg.C, cfg.N, cfg.W
    NW = C * W
    gw = (node_core[dst] * W + node_slot[dst] // P).astype(np.int64)
    # run-length (src, window) pairs
    key = src * NW + gw
    ks = np.sort(key)
    uniq, ucnt = np.unique(ks, return_counts=True)
    usrc = uniq // NW
    ugw = uniq % NW
    # per-src segment bounds in the pair arrays
    sb = np.searchsorted(usrc, np.arange(N))
    se = np.searchsorted(usrc, np.arange(N) + 1)
    deg = np.bincount(src, minlength=N)
    order = np.argsort(-deg, kind="stable")
    cnt = np.zeros((C, NW), dtype=np.int64)
    Mw = np.zeros(NW, dtype=np.int64)
    load = np.zeros(C, dtype=np.int64)
    nload = np.zeros(C, dtype=np.int64)
    cap = cfg.Np
    score = np.empty(N, dtype=np.int64)
    for n in order:
        a, b = sb[n], se[n]
        wn = ugw[a:b]
        hn = ucnt[a:b]
        cand = cnt[:, wn] + hn[None, :]
        over = np.maximum(cand - Mw[wn][None, :], 0).sum(axis=1)
        over = over.astype(np.float64) + 1e-6 * load + np.where(nload >= cap, 1e9, 0.0)
        c = int(np.argmin(over))
        score[n] = c
        cnt[c, wn] += hn
        Mw[wn] = np.maximum(Mw[wn], cnt[c, wn])
        load[c] += deg[n]
        nload[c] += 1
    # slots in assignment order per core
    sslot = np.empty(N, dtype=np.int64)
    for c in range(C):
        ns = np.where(score == c)[0]
        sslot[ns] = np.arange(len(ns))
    return score, sslot


def host_prep(cfg, features, Wd, bd, Wg, bg, Wu, bu, src, dst):
    """Returns (in_maps, node_core, node_slot, prof)."""
    C, N, Np, W = cfg.C, cfg.N, cfg.Np, cfg.W
    src = np.asarray(src).astype(np.int64)
    dst = np.asarray(dst).astype(np.int64)
    features = np.asarray(features, dtype=np.float32)

    out_deg = np.bincount(src, minlength=N)
    in_deg = np.bincount(dst, minlength=N)
    norm_src = 1.0 / np.sqrt(np.maximum(out_deg, 1.0))
    norm_dst = 1.0 / np.sqrt(np.maximum(in_deg, 1.0))

    nodes = np.arange(N)
    node_core = np.minimum(nodes // cfg.NpReal, C - 1)
    node_slot = nodes - node_core * cfg.NpReal

    # src-ownership map (where a node's h lives) is independent of the dst map
    # (which core outputs its rows); greedily balance src->core so per-window
    # edge counts are near-equal across cores (the stream pads to the max)
    score, sslot = _balance_src(cfg, src, dst, node_core, node_slot)

    # per-edge decomposition; edge owned by src's core
    ecore = score[src]
    erow = sslot[src]                          # local table row of h[src]
    dchunk = node_core[dst]
    dloc = node_slot[dst]
    dwc = dloc // P                            # window within dst chunk (0..97)
    dp = dloc % P
    gc = dwc // WPG                            # group within chunk (0..13)
    wig = dwc % WPG                            # window in group (0..6)
    # piece of each group-of-7 and group index within the piece
    gcum = np.cumsum((0,) + PIECE_G)           # [0, 7, 13, 14]
    kp_of_g = np.searchsorted(gcum[1:], np.arange(cfg.GPC), side="right")
    gip_of_g = np.arange(cfg.GPC) - gcum[kp_of_g]
    kp = kp_of_g[gc]                           # piece (0..2)
    gi = gip_of_g[gc]                          # group in (piece, chunk)
    # supergroup = program-order group index: by (piece, chunk, group)
    sg_base = np.concatenate([[0], np.cumsum([PIECE_G[k] * C for k in range(cfg.PIECES)])])
    sg = sg_base[kp] + dchunk * np.asarray(PIECE_G)[kp] + gi
    NSG = cfg.NSG

    # shared per-(sg, window) maxima
    NC = np.zeros((C, NSG, WPG), dtype=np.int64)
    np.add.at(NC, (ecore, sg, wig), 1)
    M = NC.max(axis=0)                         # [NSG, WPG]

    # blocks per supergroup chunk (stream ceil128 per sg)
    chunk_off = np.zeros(NSG, dtype=np.int64)
    chunk_len = np.zeros(NSG, dtype=np.int64)
    blocks = []       # (sg, k_in_chunk, lo, nh, bi)
    off = 0
    for s in range(NSG):
        seg = M[s]
        L = _ceil128(seg.sum())
        chunk_off[s] = off
        chunk_len[s] = L
        bcum = np.concatenate([[0], np.cumsum(seg)])
        for k in range(L // 128):
            p0, p1 = k * 128, k * 128 + 127
            lo = int(np.searchsorted(bcum[1:], p0, side="right"))
            hi = int(np.searchsorted(bcum[1:], p1, side="right"))
            lo, hi = min(lo, WPG - 1), min(hi, WPG - 1)
            nh = hi - lo + 1
            assert nh <= MAXH, f"block spans {nh} windows"
            blocks.append((s, k, lo, nh, len(blocks)))
        off += L
    T = int(off)
    NBLK = len(blocks)
    assert T == NBLK * 128

    # psum zero-region start/stop flags per (sg, 2KB region = ws//4)
    first_mm = {}
    last_mm = {}
    for s, k, lo, nh, bi in blocks:
        for h in range(nh):
            key2 = (s, (lo + h) // 4)
            if key2 not in first_mm:
                first_mm[key2] = (bi, h)
            last_mm[key2] = (bi, h)

    # gather instruction cells: greedy-pack consecutive sgs (never across pieces)
    sg_base_l = [0]
    for k in range(cfg.PIECES):
        sg_base_l.append(sg_base_l[-1] + PIECE_G[k] * C)
    gathers = []          # (o16, L)
    gather_of_sg = {}     # sg -> (cell idx, o16*16//128 = first block idx)
    piece_cells = []      # per piece: list of cell indices
    for k in range(cfg.PIECES):
        cells = []
        s0 = sg_base_l[k]
        send = sg_base_l[k + 1]
        cur = s0
        while cur < send:
            o = int(chunk_off[cur])
            L = 0
            first = cur
            while cur < send and (L == 0 or L + chunk_len[cur] <= GATHER_MAX):
                L += int(chunk_len[cur])
                cur += 1
            ci = len(gathers)
            gathers.append((o // 16, L))
            for s2 in range(first, cur):
                gather_of_sg[s2] = (ci, o // 128)
            cells.append(ci)
        piece_cells.append(tuple(cells))

    prof = {
        "chunk_off": tuple(int(x) for x in chunk_off),
        "chunk_len": tuple(int(x) for x in chunk_len),
        "blocks": tuple(blocks),
        "first": frozenset(first_mm.items()),
        "last": frozenset(last_mm.items()),
        "gathers": tuple(gathers),
        "gather_of_sg": tuple(sorted(gather_of_sg.items())),
        "piece_cells": tuple(tuple(x) for x in piece_cells),
        "sg_base": tuple(sg_base_l),
        "T": T,
        "NBLK": NBLK,
    }

    # fused weights
    Wgu = (np.asarray(Wg, np.float64) @ np.asarray(Wu, np.float64)).astype(np.float32)
    bu2 = (np.asarray(bg, np.float64) @ np.asarray(Wu, np.float64) + bu).astype(np.float32)

    wd_h = np.zeros((P, cfg.KIN * cfg.OUT), dtype=np.float16)
    for cc in range(cfg.KC):
        wd_h[:, cc * cfg.OUT:(cc + 1) * cfg.OUT] = Wd[cc * P:(cc + 1) * P, :]
    wd_h[0, cfg.KC * cfg.OUT:(cfg.KC + 1) * cfg.OUT] = bd
    wgu_h = Wgu.astype(np.float16)

    consts = np.zeros((P, IOTA_W), dtype=np.float16)
    consts[:, :] = np.arange(IOTA_W, dtype=np.float16)[None, :]

    # per-block lo for slot_rel
    blk_lo = np.zeros(NBLK, dtype=np.int64)
    for s, k, lo, nh, bi in blocks:
        assert chunk_off[s] // 128 + k == bi
        blk_lo[bi] = lo

    # intra-chunk window segment offsets (shared)
    segoff = np.zeros((NSG, WPG), dtype=np.int64)
    for s in range(NSG):
        segoff[s] = chunk_off[s] + np.concatenate([[0], np.cumsum(M[s])[:-1]])

    S16 = T // 16
    in_maps = []
    for c in range(C):
        em = np.where(ecore == c)[0]
        sgc, wigc, dpc = sg[em], wig[em], dp[em]
        order = np.lexsort((dpc, wigc, sgc))
        em, sgc, wigc, dpc = em[order], sgc[order], wigc[order], dpc[order]
        segid = sgc * WPG + wigc
        uniq, counts = np.unique(segid, return_counts=True)
        cum = np.concatenate([[0], np.cumsum(counts)])
        rank = np.arange(len(em)) - cum[np.searchsorted(uniq, segid)]
        pos = segoff[sgc, wigc] + rank
        assert (rank < M[sgc, wigc]).all()

        idx_s = np.zeros(T, dtype=np.int64)
        slotg = np.full(T, -1, dtype=np.int64)   # slot within group (wig*128+p)
        idx_s[pos] = erow[em]
        slotg[pos] = wigc * P + dpc

        # per-block relative slots
        slot_rel = slotg.reshape(-1, 128) - blk_lo[:, None] * P
        slot_rel[slotg.reshape(-1, 128) < 0] = -1
        assert (slot_rel < MAXH * P).all()

        idx16 = np.zeros((P, S16), dtype=np.int16)
        sidx = np.arange(S16) * 16
        for p in range(P):
            idx16[p, :] = idx_s[sidx + (p % 16)]
        slotv = np.ascontiguousarray(slot_rel.T.astype(np.float32))   # [128, NBLK]

        # xa: window-blocked [p, w*KIN*128 + cc*128 + n]; src-owned nodes,
        # norm_src folded
        nt_ids = np.where(score == np.int64(c))[0]
        xs = (features[nt_ids, :] * norm_src[nt_ids, None]).astype(np.float16)
        xa = np.zeros((P, W * cfg.KIN * P), dtype=np.float16)
        xs_slot = np.zeros((Np, cfg.IN), dtype=np.float16)
        xs_slot[sslot[nt_ids], :] = xs
        nsr = np.zeros(Np, dtype=np.float16)
        nsr[sslot[nt_ids]] = norm_src[nt_ids].astype(np.float16)
        for w in range(W):
            blkb = w * cfg.KIN * P
            rows = xs_slot[w * P:(w + 1) * P, :]
            for cc in range(cfg.KC):
                xa[:, blkb + cc * P:blkb + (cc + 1) * P] = rows[:, cc * P:(cc + 1) * P].T
            xa[0, blkb + cfg.KC * P:blkb + (cfg.KC + 1) * P] = nsr[w * P:(w + 1) * P]

        # own-chunk norm_dst per (partition, window) -- keyed by the dst map
        dt_ids = np.where(node_core == np.int64(c))[0]
        ndstw = np.zeros((P, W), dtype=np.float32)
        nd = np.zeros(Np, dtype=np.float32)
        nd[node_slot[dt_ids]] = norm_dst[dt_ids].astype(np.float32)
        ndstw[:, :] = nd.reshape(W, P).T

        in_maps.append(
            {
                "xa": xa,
                "idx": idx16,
                "slotv": slotv,
                "wd": wd_h,
                "wgu": wgu_h,
                "consts": consts,
                "ndstw": ndstw,
            }
        )

    return in_maps, node_core, node_slot, prof


def build_graph(cfg, prof):
    """Build the SPMD Bass graph (same for all cores)."""
    W, OUT, IN = cfg.W, cfg.OUT, cfg.IN
    C, PIECES = cfg.C, cfg.PIECES
    blocks = prof["blocks"]
    first_mm = dict(prof["first"])
    last_mm = dict(prof["last"])
    gathers = prof["gathers"]
    gather_of_sg = dict(prof["gather_of_sg"])
    piece_cells = prof["piece_cells"]
    sg_base = prof["sg_base"]
    T = prof["T"]
    NBLK = prof["NBLK"]
    S16 = T // 16

    blocks_by_sg = {}
    for b in blocks:
        blocks_by_sg.setdefault(b[0], []).append(b)

    nc = bacc.Bacc(None, target_bir_lowering=False)
    xa = nc.declare_dram_parameter("xa", [P, W * cfg.KIN * P], F16, False)
    idx = nc.declare_dram_parameter("idx", [P, S16], I16, False)
    slotv = nc.declare_dram_parameter("slotv", [P, NBLK], F32, False)
    wd = nc.declare_dram_parameter("wd", [P, cfg.KIN * OUT], F16, False)
    wgu = nc.declare_dram_parameter("wgu", [OUT, IN], F16, False)
    consts = nc.declare_dram_parameter("consts", [P, IOTA_W], F16, False)
    ndstw = nc.declare_dram_parameter("ndstw", [P, W], F32, False)
    out = nc.declare_dram_parameter("out", [cfg.Np, IN], F16, True)

    with tile.TileContext(nc) as tc:
        with (
            tc.tile_pool(name="dram", bufs=1, space="DRAM") as dram,
            tc.tile_pool(name="gpsum", bufs=2, space="PSUM") as gpsum,
            tc.tile_pool(name="upsum", bufs=2, space="PSUM") as upsum,
            tc.tile_pool(name="bconst", bufs=1) as bconst,
        ):
            htab = dram.tile([cfg.Np, OUT], F16)
            ptabs = []
            rsouts = []
            for k in range(PIECES):
                pt = dram.tile([C, P, cfg.piece_nw[k] * P], F16, name=f"ptab{k}")
                ro = dram.tile([P, cfg.piece_nw[k] * P], F16, name=f"rsout{k}")
                ptabs.append(pt)
                rsouts.append(ro)
            nc.gpsimd.load_library(library_config.mlp)

            # ---- phase A: down-projection into the local DRAM h table ----
            with (
                tc.tile_pool(name="aconst", bufs=1) as aconst,
                tc.tile_pool(name="xat", bufs=3) as xap,
                tc.tile_pool(name="hst", bufs=1) as hstp,
            ):
                wd_sb = aconst.tile([P, cfg.KIN * OUT], F16)
                nc.sync.dma_start(out=wd_sb[:], in_=wd[:, :])
                h_stage = hstp.tile([P, W * OUT], F16)
                xa_v = xa[:, :].rearrange("p (w x) -> p w x", w=W)
                qbounds = [0]
                step0 = max(4, W // 12)
                qbounds.append(min(step0, W))
                while qbounds[-1] < W:
                    qbounds.append(min(qbounds[-1] + (W + 3) // 4, W))
                for qd in range(len(qbounds) - 1):
                    lo = qbounds[qd]
                    hi = qbounds[qd + 1]
                    xt = xap.tile([P, (hi - lo) * cfg.KIN * P], F16)
                    xa_eng = nc.sync if qd % 2 == 0 else nc.scalar
                    xa_eng.dma_start(
                        out=xt[:].rearrange("p (w x) -> p w x", w=hi - lo),
                        in_=xa_v[:, lo:hi, :],
                    )
                    for w in range(lo, hi):
                        blkb = (w - lo) * cfg.KIN * P
                        psfull = gpsum.tile([P, (WPG + 1) * P], F32, space="PSUM", tag="PS")
                        ps = psfull[:, 0:OUT]
                        for ccx in range(cfg.KC):
                            nc.tensor.matmul(
                                ps[:],
                                lhsT=xt[:, blkb + ccx * P:blkb + (ccx + 1) * P],
                                rhs=wd_sb[:, ccx * OUT:(ccx + 1) * OUT],
                                start=(ccx == 0),
                                stop=False,
                            )
                        nc.tensor.matmul(
                            ps[:],
                            lhsT=xt[0:1, blkb + cfg.KC * P:blkb + (cfg.KC + 1) * P],
                            rhs=wd_sb[0:1, cfg.KC * OUT:cfg.KC * OUT + OUT],
                            start=False,
                            stop=True,
                        )
                        nc.vector.tensor_copy(
                            out=h_stage[:, w * OUT:(w + 1) * OUT], in_=ps[:]
                        )
                    nc.scalar.dma_start(
                        out=htab[lo * P:hi * P, :].rearrange("(w p) f -> p w f", p=P),
                        in_=h_stage[:, lo * OUT:hi * OUT].rearrange(
                            "p (w f) -> p w f", w=hi - lo
                        ),
                    )

            # phase-B constants: issued after the xa loads so the first
            # gather's idx data rides the DMA rails during the down-proj tail
            wgu_sb = bconst.tile([OUT, IN], F16)
            nc.sync.dma_start(out=wgu_sb[:], in_=wgu[:, :])
            cst = bconst.tile([P, IOTA_W], F16)
            nc.sync.dma_start(out=cst[:], in_=consts[:, :])
            ndst_sb = bconst.tile([P, W], F32)
            nc.sync.dma_start(out=ndst_sb[:], in_=ndstw[:, :])
            idx_sb = bconst.tile([P, S16], I16)
            nc.scalar.dma_start(out=idx_sb[:], in_=idx[:, :])
            slot_sb = bconst.tile([P, NBLK], F32)
            nc.scalar.dma_start(out=slot_sb[:], in_=slotv[:, :])

            # ---- phase B: src-side aggregation, piece-wise RS, up-projection ----
            with (
                tc.tile_pool(name="gp", bufs=3) as gp,
                tc.tile_pool(name="sp", bufs=12) as sp,
                tc.tile_pool(name="stg", bufs=4) as stgp,
                tc.tile_pool(name="agr", bufs=2) as agrp,
                tc.tile_pool(name="osb", bufs=2) as osb,
            ):
                cell_tiles = {}

                def emit_gathers(k):
                    for ci in piece_cells[k]:
                        o16, L = gathers[ci]
                        gt = gp.tile([P, (L // 128) * OUT], F16, tag="G", name=f"g{ci}")
                        nc.gpsimd.dma_gather(
                            out_ap=gt[:].rearrange("p (b f) -> p b f", b=L // 128),
                            in_ap=htab[:, :],
                            idxs_ap=idx_sb[:, o16:o16 + L // 16],
                            num_idxs=L,
                            num_idxs_reg=L,
                            elem_size=OUT,
                            single_packet=False,
                        )
                        cell_tiles[ci] = gt

                def emit_agg(k):
                    for s in range(sg_base[k], sg_base[k + 1]):
                        ci, b0 = gather_of_sg[s]
                        gt = cell_tiles[ci]
                        c = (s - sg_base[k]) // PIECE_G[k]
                        gi = (s - sg_base[k]) % PIECE_G[k]
                        psg = gpsum.tile([P, (WPG + 1) * P], F32, space="PSUM", tag="PS")
                        for s2, kb, lo, nh, bi in blocks_by_sg[s]:
                            S = sp.tile([P, nh * P], F16, tag="S")
                            nc.vector.tensor_scalar(
                                out=S[:],
                                in0=cst[:, 0:nh * P],
                                scalar1=slot_sb[:, bi:bi + 1],
                                scalar2=None,
                                op0=mybir.AluOpType.is_equal,
                            )
                            for h in range(nh):
                                ws = lo + h
                                nc.tensor.matmul(
                                    psg[:, ws * P:(ws + 1) * P],
                                    lhsT=gt[:, (bi - b0) * OUT:(bi - b0 + 1) * OUT],
                                    rhs=S[:, h * P:(h + 1) * P],
                                    start=(first_mm.get((s, ws // 4)) == (bi, h)),
                                    stop=(last_mm.get((s, ws // 4)) == (bi, h)),
                                )
                        stg = stgp.tile([P, WPG * P], F16, tag="D")
                        nc.scalar.activation(
                            out=stg[:],
                            in_=psg[:, 0:WPG * P],
                            func=mybir.ActivationFunctionType.Copy,
                        )
                        nc.sync.dma_start(
                            out=ptabs[k][c, :, gi * WPG * P:(gi + 1) * WPG * P],
                            in_=stg[:],
                        )

                def emit_rs(k):
                    # on the Pool queue (walrus requires Pool for collectives);
                    # issued AFTER piece k+1's gathers so it never stalls them
                    nc.gpsimd.collective_compute(
                        "ReduceScatter",
                        mybir.AluOpType.add,
                        replica_groups=[list(range(C))],
                        ins=[ptabs[k][:].opt()],
                        outs=[rsouts[k][:].opt()],
                    )

                def emit_upproj(k):
                    nw = cfg.piece_nw[k]
                    ld_eng = nc.scalar if k % 2 == 0 else nc.sync
                    aggR = agrp.tile([P, nw * P], F16, tag="A", name=f"ar{k}")
                    ld_eng.dma_start(out=aggR[:], in_=rsouts[k][:, :])
                    for gi2 in range(PIECE_G[k]):
                        ost = osb.tile([P, WPG * IN], F16, tag="O")
                        for wl in range(WPG):
                            w = cfg.piece_w0[k] + gi2 * WPG + wl
                            ps2 = upsum.tile([P, IN], F32, space="PSUM", tag="UP")
                            for lo2 in range(0, IN, 512):
                                hi2 = min(lo2 + 512, IN)
                                nc.tensor.matmul(
                                    ps2[:, lo2:hi2],
                                    lhsT=aggR[:, (gi2 * WPG + wl) * P:(gi2 * WPG + wl + 1) * P],
                                    rhs=wgu_sb[:, lo2:hi2],
                                    start=True,
                                    stop=True,
                                )
                            if k != 1:
                                nc.scalar.activation(
                                    out=ost[:, wl * IN:(wl + 1) * IN],
                                    in_=ps2[:],
                                    func=mybir.ActivationFunctionType.Copy,
                                    scale=ndst_sb[:, w:w + 1],
                                )
                            else:
                                nc.vector.tensor_scalar(
                                    out=ost[:, wl * IN:(wl + 1) * IN],
                                    in0=ps2[:],
                                    scalar1=ndst_sb[:, w:w + 1],
                                    scalar2=None,
                                    op0=mybir.AluOpType.mult,
                                )
                        w0 = cfg.piece_w0[k] + gi2 * WPG
                        out_eng = nc.scalar if k % 2 == 0 else nc.sync
                        out_eng.dma_start(
                            out=out[w0 * P:(w0 + WPG) * P, :].rearrange(
                                "(w p) f -> p w f", p=P
                            ),
                            in_=ost[:].rearrange("p (w f) -> p w f", w=WPG),
                        )

                for k in range(PIECES):
                    emit_gathers(k)
                for k in range(PIECES):
                    emit_agg(k)
                    emit_rs(k)
                    if k >= 1:
                        emit_upproj(k - 1)
                emit_upproj(PIECES - 1)

    nc.compile()
    return nc


_GRAPH_CACHE = {}


def kernel(features, Wd, bd, Wg, bg, Wu, bu, src, dst):
    features = np.asarray(features)
    N, IN = features.shape
    OUT = np.asarray(Wd).shape[1]
    E = np.asarray(src).shape[0]
    cfg = Cfg(N, E, IN, OUT)

    in_maps, node_core, node_slot, prof = host_prep(
        cfg, features, Wd, bd, Wg, bg, Wu, bu, src, dst
    )
    key = (N, E, IN, OUT, prof["T"], prof["blocks"])
    nc = _GRAPH_CACHE.get(key)
    if nc is None:
        nc = build_graph(cfg, prof)
        _GRAPH_CACHE[key] = nc

    res = run_bass_kernel_spmd(nc, in_maps, core_ids=list(range(cfg.C)))
    allo = np.stack([np.asarray(res.results[i]["out"]) for i in range(cfg.C)])
    out = allo[node_core, node_slot, :].astype(np.float32)
    bu2 = (np.asarray(bg, np.float64) @ np.asarray(Wu, np.float64) + bu).astype(
        np.float32
    )
    out += bu2[None, :]
    return out
